# revision 4
# baseline (speedup 1.0000x reference)
"""Trainium2 Bass kernel for nn_GroupLocalSL2 — 1D-Winograd F(4,5) along width.

out[b,o,i,r,y] = sum_{c,f,kh,kw} x[b,c,idx[i,f],r+kh,y+kw] * W[o,c,f,kh,kw] + bias[o]

Width axis y is Winograd-transformed with F(4,5): 8 points u, 15 tiles s of 4
output cols each.  Per point u the contraction over (c,f,kh) is a standard
kh-accumulated GEMM — 2.5x fewer PE column-streams than direct conv.

Pipeline (per core, batch b):
  Phase T (once per input group g, 33 groups of 4-group quad tiles):
    x uploaded host-pre-transposed as xT2_q[(rowpar,col), (c,gq)] per row-pair q
    stage1 matmul: lhsT=xT2_q (stationary), rhs=B2 (banded Winograd B^T const)
      -> psum[(c,gq), (rowpar,u,s)] = Xhat
    copy psum->sbuf bf16; 3 most-used quad tiles stay SBUF-resident, the rest
    are staged to DRAM scratch xhat_d[g, c, row, u, s].
  Main loop per (i, row-chunk rc in {0:32, 32:28}):
    gather: 7 DMAs xhat_d[idx[i,f]] -> Xa[(c,f0..3)=128,...], Xb[(c,f4..6)=96,...]
    GEMM: per point-pair k (psum col-tiled M=64 halves), accumulate
      2 K-chunks x 5 kh matmuls -> Mt psum [(2pts,o), r, s]
    copy Mt -> sbuf bf16; inverse transform = 4 accumulating matmuls per j-pair
      with block-diag A^T constants -> psum [(j-parity,o), r, s]
    bias-add + j-deinterleave (ACT/DVE) -> ot[o, r, s, j] fp32, DMA out.

Batch B=8 data-parallel across 8 cores.  idx baked into the DMA program.
"""

import os
import sys

import numpy as np
import ml_dtypes

for _p in ("/opt/trn_rl_repo", "/root/.axon_site/_ro/trn_rl_repo"):
    if os.path.isdir(_p) and _p not in sys.path:
        sys.path.append(_p)

import concourse.bass as bass
import concourse.mybir as mybir
import concourse.tile as tile
from concourse import bacc
from concourse.bass_utils import run_bass_kernel_spmd

BF16 = ml_dtypes.bfloat16

B, C, G_IN = 8, 32, 33
O, G_F, KH, KW = 64, 7, 5, 5
X, Y = 64, 64
G_OUT = 15
XO, YO = 60, 60
NU, NS = 8, 15          # Winograd points / width tiles
US = NU * NS            # 120
RCH = [(0, 32), (32, 28)]

# F(4,5) Cook-Toom matrices, nodes {0, 1, -1, 2, -2, 1/2, -1/2, inf}.
# All entries are exactly representable in bf16.
AT_M = np.array([
    [1, 1,  1, 1,  1, 1,     1,     0],
    [0, 1, -1, 2, -2, 0.5,  -0.5,   0],
    [0, 1,  1, 4,  4, 0.25,  0.25,  0],
    [0, 1, -1, 8, -8, 0.125, -0.125, 1],
], dtype=np.float64)                     # [4, 8]
G_M = np.array([
    [-1, 0, 0, 0, 0],
    [-2/9, -2/9, -2/9, -2/9, -2/9],
    [-2/9, 2/9, -2/9, 2/9, -2/9],
    [1/90, 2/90, 4/90, 8/90, 16/90],
    [1/90, -2/90, 4/90, -8/90, 16/90],
    [64/90, 32/90, 16/90, 8/90, 4/90],
    [64/90, -32/90, 16/90, -8/90, 4/90],
    [0, 0, 0, 0, 1],
], dtype=np.float64)                     # [8, 5]
BT_M = np.array([
    [-1,  0,  5.25,  0,   -5.25,  0,    1, 0],
    [ 0,  1,  1,    -4.25, -4.25, 1,    1, 0],
    [ 0, -1,  1,     4.25, -4.25, -1,   1, 0],
    [ 0,  0.5, 0.25, -2.5, -1.25, 2,    1, 0],
    [ 0, -0.5, 0.25,  2.5, -1.25, -2,   1, 0],
    [ 0,  2,  4,    -2.5,  -5,    0.5,  1, 0],
    [ 0, -2,  4,     2.5,  -5,   -0.5,  1, 0],
    [ 0, -1,  0,     5.25,  0,   -5.25, 0, 1],
], dtype=np.float64)                     # [8, 8]


def _build_nc(idx):
    nc = bacc.Bacc("TRN2", target_bir_lowering=False, debug=False)
    dt = mybir.dt
    # x pre-transposed host-side: x_d[t, rp*64+col, q, gq*32+c] = x[c, 4t+gq, 2q+rp, col]
    x_d = nc.dram_tensor("x", [9, 128, 32, 128], dt.bfloat16, kind="ExternalInput")
    wa_d = nc.dram_tensor("wa", [128, KH, NU, O], dt.bfloat16, kind="ExternalInput")
    wb_d = nc.dram_tensor("wb", [96, KH, NU, O], dt.bfloat16, kind="ExternalInput")
    b2_d = nc.dram_tensor("b2", [128, 2 * US], dt.bfloat16, kind="ExternalInput")
    ak_d = nc.dram_tensor("ak", [8, 128, 128], dt.bfloat16, kind="ExternalInput")
    bias_d = nc.dram_tensor("bias", [O, 1], dt.float32, kind="ExternalInput")
    out_d = nc.dram_tensor("out", [O, G_OUT, XO, YO], dt.float32, kind="ExternalOutput")

    with tile.TileContext(nc) as tc:
        with (
            tc.tile_pool(name="consts", bufs=1) as cpool,
            tc.tile_pool(name="xt", bufs=3) as xtpool,
            tc.tile_pool(name="stage", bufs=2) as stpool,
            tc.tile_pool(name="gather", bufs=2) as gpool,
            tc.tile_pool(name="msb", bufs=1) as mpool,
            tc.tile_pool(name="osb", bufs=2) as opool,
            tc.tile_pool(name="dram", bufs=1, space="DRAM") as dpool,
        ):
            b2 = cpool.tile([128, 2 * US], dt.bfloat16, tag="b2")
            wa = cpool.tile([128, KH, NU, O], dt.bfloat16, tag="wa")
            wb = cpool.tile([96, KH, NU, O], dt.bfloat16, tag="wb")
            ak = cpool.tile([128, 8, 128], dt.bfloat16, tag="ak")
            bias_sb = cpool.tile([O, 1], dt.float32, tag="bias")
            nc.sync.dma_start(b2[:, :], b2_d[:, :])
            nc.scalar.dma_start(wa[:, :, :, :], wa_d[:, :, :, :])
            nc.scalar.dma_start(wb[:, :, :, :], wb_d[:, :, :, :])
            nc.scalar.dma_start(bias_sb[:, :], bias_d[:, :])
            for slot in range(8):
                nc.scalar.dma_start(ak[:, slot, :], ak_d[slot, :, :])

            xhat_d = dpool.tile([G_IN, C, X, US], dt.bfloat16, tag="xhat")

            # ---- Phase T: width-transform every input group ----
            # Keep the NSB most-used quad tiles SBUF-resident (gathered via
            # on-chip SBUF->SBUF DMA); the rest round-trip DRAM.
            NSB = 3
            usage = [0] * 9
            for i in range(G_OUT):
                for f in range(G_F):
                    usage[int(idx[i, f]) // 4] += 1
            resident = set(sorted(range(9), key=lambda t: -usage[t])[:NSB])
            xsb = {
                t: cpool.tile([128, 32, 2 * US], dt.bfloat16, tag=f"xsb{t}",
                              name=f"xsb{t}")
                for t in sorted(resident)
            }
            NPRE = 2
            need_t = {}
            for i in range(NPRE):
                for f in range(G_F):
                    t = int(idx[i, f]) // 4
                    need_t.setdefault(t, 9 + t)
                    need_t[t] = min(need_t[t], i)
            t_order = sorted(range(9), key=lambda t: (need_t.get(t, 99), t))
            # gather tiles per (chunk, rc): rows r0..r0+35
            GR = 36

            def mk_gtiles(i):
                return [
                    (
                        gpool.tile([128, GR, NU, NS], dt.bfloat16, tag=f"ga{rci}",
                                   name=f"ga{rci}_{i}"),
                        gpool.tile([96, GR, NU, NS], dt.bfloat16, tag=f"gb{rci}",
                                   name=f"gb{rci}_{i}"),
                    )
                    for rci in range(2)
                ]

            gtiles = {i: mk_gtiles(i) for i in range(NPRE)}

            def emit_gather(i, f, rci, tiles, eng=None):
                eng = eng or nc.sync
                g = int(idx[i, f])
                t, gq = g // 4, g % 4
                r0, R = RCH[rci]
                nrow = min(GR, X - r0)
                xa, xb = tiles[rci]
                if f < 4:
                    dst = xa[32 * f : 32 * f + 32, 0:nrow, :, :]
                else:
                    dst = xb[32 * (f - 4) : 32 * (f - 4) + 32, 0:nrow, :, :]
                if t in resident:
                    src = (
                        xsb[t][32 * gq : 32 * gq + 32,
                               r0 // 2 : r0 // 2 + nrow // 2, :]
                        .rearrange("p q (rp u s) -> p (q rp) u s", rp=2, u=NU)
                    )
                else:
                    src = xhat_d[g, :, r0 : r0 + nrow, :].rearrange(
                        "p r (u s) -> p r u s", u=NU
                    )
                eng.dma_start(dst, src)

            with tc.tile_pool(name="psT", bufs=4, space="PSUM") as ppT:
                for t in t_order:
                    ng = 4 if t < 8 else 1
                    P = 32 * ng
                    if t in resident:
                        st = xsb[t]
                    else:
                        st = stpool.tile([128, 32, 2 * US], dt.bfloat16, tag="st")
                    xtt = xtpool.tile([128, 32, 128], dt.bfloat16, tag="xtt")
                    nc.sync.dma_start(xtt[:, 0:16, 0:P], x_d[t, :, 0:16, 0:P])
                    nc.sync.dma_start(xtt[:, 16:32, 0:P], x_d[t, :, 16:32, 0:P])
                    for q0 in range(0, 32, 4):
                        ps = ppT.tile([128, 4, 256], dt.float32, tag="pT")
                        for qq in range(4):
                            nc.tensor.matmul(
                                ps[0:P, qq, 0 : 2 * US],
                                xtt[:, q0 + qq, 0:P], b2[:, :],
                                start=True, stop=True,
                            )
                        if q0 % 8 == 0:
                            nc.scalar.copy(
                                st[0:P, q0 : q0 + 4, :], ps[0:P, :, 0 : 2 * US]
                            )
                        else:
                            nc.vector.tensor_scalar_add(
                                st[0:P, q0 : q0 + 4, :], ps[0:P, :, 0 : 2 * US], 0.0
                            )
                    if t not in resident:
                        nc.sync.dma_start(
                            xhat_d[4 * t : 4 * t + ng, :, :, :],
                            st[0:P, :, :],
                        )
                    # prefetch gathers for i<NPRE whose group was just done
                    for i in range(NPRE):
                        for f in range(G_F):
                            if int(idx[i, f]) // 4 == t:
                                for rci in range(2):
                                    emit_gather(i, f, rci, gtiles[i],
                                                eng=nc.gpsimd)

            # ---- Main loop ----
            with (
                tc.tile_pool(name="psG", bufs=2, space="PSUM") as ppG,
                tc.tile_pool(name="psJ", bufs=2, space="PSUM") as ppJ,
            ):
                for i in range(G_OUT):
                    if i in gtiles:
                        tiles = gtiles[i]
                    else:
                        tiles = mk_gtiles(i)
                        for f in range(G_F):
                            for rci in range(2):
                                emit_gather(i, f, rci, tiles)

                    msbs = {}
                    for k in range(4):
                        pk = {}
                        for rci, (r0, R) in enumerate(RCH):
                            pk[rci] = ppG.tile(
                                [128, 32, NS], dt.float32,
                                tag=f"rc{rci}", name=f"pk{k}_{rci}",
                            )
                        for ci in range(2):
                            wg, Kc = (wa, 128) if ci == 0 else (wb, 96)
                            for kh in range(KH):
                                for rci, (r0, R) in enumerate(RCH):
                                    xg = tiles[rci][ci]
                                    for par in range(2):
                                        u = 2 * k + par
                                        nc.tensor.matmul(
                                            pk[rci][64 * par : 64 * par + 64, 0:R, :],
                                            wg[0:Kc, kh, u, :],
                                            xg[0:Kc, kh : kh + R, u, :],
                                            start=(ci == 0 and kh == 0),
                                            stop=(ci == 1 and kh == KH - 1),
                                        )
                        for rci, (r0, R) in enumerate(RCH):
                            m = mpool.tile(
                                [128, 32, NS], dt.bfloat16,
                                tag=f"m{k}_{rci}", name=f"m{k}_{rci}",
                            )
                            if (k + rci) % 2 == 0:
                                nc.scalar.copy(m[:, 0:R, :], pk[rci][:, 0:R, :])
                            else:
                                nc.vector.tensor_scalar_add(
                                    m[:, 0:R, :], pk[rci][:, 0:R, :], 0.0
                                )
                            msbs[(k, rci)] = m

                    for rci, (r0, R) in enumerate(RCH):
                        pjs = [
                            ppJ.tile([128, 32, NS], dt.float32,
                                     tag=f"pj{jp}", name=f"pj{jp}")
                            for jp in range(2)
                        ]
                        for jp in range(2):
                            for k in range(4):
                                nc.tensor.matmul(
                                    pjs[jp][:, 0:R, :],
                                    ak[:, 2 * k + jp, :],
                                    msbs[(k, rci)][:, 0:R, :],
                                    start=(k == 0), stop=(k == 3),
                                )
                        ot = opool.tile([O, 32, NS, 4], dt.float32, tag="ot")
                        for jp in range(2):
                            for jj in range(2):
                                j = 2 * jp + jj
                                src = pjs[jp][64 * jj : 64 * jj + 64, 0:R, :]
                                if j % 2 == 0:
                                    nc.scalar.add(
                                        ot[:, 0:R, :, j], src, bias_sb[:, 0:1]
                                    )
                                else:
                                    nc.vector.tensor_scalar_add(
                                        ot[:, 0:R, :, j], src, bias_sb[:, 0:1]
                                    )
                        nc.gpsimd.dma_start(
                            out_d[:, i, r0 : r0 + R, :], ot[:, 0:R, :, :]
                        )
    nc.compile()
    return nc


def _prep_inputs(x, weight, bias, idx):
    x16 = np.asarray(x).astype(np.float32).astype(BF16)  # [B, C, G_IN, X, Y]
    w = np.asarray(weight).astype(np.float64)            # [O, C, G_F, KH, KW]

    # Winograd weight transform: Wh[u,o,c,f,kh] = sum_kw G[u,kw] * w[o,c,f,kh,kw]
    Wh = np.einsum("uk,ocfhk->uocfh", G_M, w)
    wfull = np.ascontiguousarray(Wh.transpose(3, 2, 4, 0, 1))  # [f, c, kh, u, o]
    wfull = wfull.reshape(G_F * C, KH, NU, O).astype(BF16)
    wa = np.ascontiguousarray(wfull[0:128])
    wb = np.ascontiguousarray(wfull[128:224])

    # Banded B^T: b2[rp*64+col, rp2*120+u*15+s] = (rp==rp2)*BT[u, col-4s]
    b2 = np.zeros((128, 2 * US), np.float64)
    for rp in range(2):
        for col in range(Y):
            for s in range(NS):
                b = col - 4 * s
                if 0 <= b < 8:
                    for u in range(NU):
                        b2[rp * 64 + col, rp * US + u * NS + s] = BT_M[u, b]
    b2 = b2.astype(BF16)

    # Inverse-transform constants: ak[k*2+jp, up*64+o, jj*64+o'] =
    #   AT[2jp+jj, 2k+up] * (o==o')
    akm = np.zeros((8, 128, 128), np.float64)
    eye = np.eye(O)
    for k in range(4):
        for jp in range(2):
            for up in range(2):
                for jj in range(2):
                    akm[2 * k + jp, 64 * up : 64 * up + 64, 64 * jj : 64 * jj + 64] = (
                        AT_M[2 * jp + jj, 2 * k + up] * eye
                    )
    akm = akm.astype(BF16)

    b2c = np.ascontiguousarray(b2)
    akc = np.ascontiguousarray(akm)
    bias2 = np.ascontiguousarray(np.asarray(bias).astype(np.float32).reshape(O, 1))
    in_maps = []
    for b in range(B):
        # xT[t, rp*64+col, q, gq*32+c] = x[c, 4t+gq, 2q+rp, col]
        xb = x16[b]                                   # [C, G_IN, X, Y]
        tmp = np.zeros((C, 36, X, Y), dtype=BF16)
        tmp[:, :G_IN] = xb
        tmp = tmp.reshape(C, 9, 4, 32, 2, Y)          # c, t, gq, q, rp, y
        xT = np.ascontiguousarray(
            tmp.transpose(1, 4, 5, 3, 2, 0)           # t, rp, y, q, gq, c
        ).reshape(9, 128, 32, 128)
        in_maps.append(
            {"x": xT, "wa": wa, "wb": wb, "b2": b2c, "ak": akc, "bias": bias2}
        )
    return in_maps


def run(x, weight, bias, idx, trace=False):
    idx = np.asarray(idx).astype(np.int64)
    assert idx.shape == (G_OUT, G_F) and idx.min() >= 0 and idx.max() < G_IN
    nc = _build_nc(idx)
    in_maps = _prep_inputs(x, weight, bias, idx)
    res = run_bass_kernel_spmd(nc, in_maps, list(range(B)), trace=trace)
    out = np.stack([res.results[b]["out"] for b in range(B)]).astype(np.float32)
    return out, res


def kernel(x, weight, bias, idx):
    out, _ = run(x, weight, bias, idx, trace=False)
    return out
